# revision 6
# baseline (speedup 1.0000x reference)
"""Trainium2 Bass kernel for coverage (Bahdanau-style) attention.

Reference computation (B=32, S=2048, H=1024):
    enc_feature = encoder_outputs @ W_enc.T                    # [B,S,H]
    dec_feature = decoder_hidden @ W_dec.T + b_dec             # [B,1,H]
    cov_feature = coverage[..., None] * w_cov                  # [B,S,H]
    scores      = tanh(enc_feature + dec_feature + cov_feature)
    attn_scores = scores @ v                                   # [B,S]
    attn_dist   = softmax(attn_scores, axis=-1)[:, None, :]    # [B,1,S]

Sharding: data-parallel over batch B across 8 cores (4 batches/core).

Per-core device kernel — fp8 DoubleRow hi/lo scheme:
  - The main matmul runs in fp8e4 (e4m3) with MatmulPerfMode.DoubleRow,
    which processes TWO 128-row contraction subtiles per instruction at
    0.5 cycles per output column. To keep bf16-grade accuracy, each operand
    is split hi/lo with error feedback (x = Q8(x) + Q8(x - Q8(x))) and the
    product is built from three DoubleRow passes per k-subtile-pair:
    eh@Wh + el@Wh + eh@Wl (the el@Wl cross term is negligible). Total PE
    cost is 0.75x the fp32r cycle count. Measured end-to-end rel err vs
    the fp32 reference: ~2e-3 (gate is 2e-2).
  - W is pre-scaled by 32 on the host so Wl stays out of fp8 subnormal
    underflow; the tanh activation applies scale=1/32 to compensate (the
    coverage weight w_cov is pre-scaled x32 on the host for the same
    reason).
  - dec_feature (+b_dec) is computed on host and fused as the tanh
    per-partition bias. The coverage rank-1 term is fused into PSUM by one
    DVE scalar_tensor_tensor (pre = bc * wcov_m + psum, in place).
  - The v-dot no longer uses the PE at all (it wasted 12.5% of PE cycles
    at 1/128 utilization): tanh output tt (bf16) is multiply-accumulated
    per h-chunk into two f32 accumulators, four chunks on DVE and four on
    GPSIMD, merged, then summed across partitions with
    gpsimd.partition_all_reduce.
  - softmax per batch row: exp on ScalarE, partial sums + normalize on DVE
    (no max subtraction needed: |scores| <= sum|v| ~ 25, exp safe in f32).
  - PE warmup matmuls fill the initial DMA window (keeps the p-state
    clock ramp warm so real matmuls run at 2.4 GHz).

Engine budget per 512-row block (16 blocks/core): PE 96 DoubleRow matmuls
= 24576 cycles ~ 10.3us; ACT 8 tanh + 1 exp ~ 5.5us; DVE ~ 6.5us; Pool
~ 4.8us. PE-bound at ~94% occupancy.
"""

import os

# The device path runs through jax/PJRT on the axon-tunneled NeuronCores;
# make sure the axon platform is preferred if nothing else was configured.
os.environ.setdefault("JAX_PLATFORMS", "axon,cpu")

import ml_dtypes
import numpy as np

import concourse.bass as bass
import concourse.bass_isa as bass_isa
import concourse.mybir as mybir
import concourse.tile as tile
from concourse import bacc
from concourse.bass_utils import run_bass_kernel_spmd

B, S, H = 32, 2048, 1024
NCORES = 8
BC = B // NCORES          # batches per core
R = BC * S                # rows per core
P = 128
NF = 512                  # matmul moving free dim / row-block size
KC = H // P               # contraction subtiles of 128
MC = H // P               # h_out chunks
NRB = R // NF             # row blocks per core
RB_PER_B = S // NF        # row blocks per batch
ALPHA = 32.0              # host-side W scale (undone by tanh scale=1/32)

F32 = mybir.dt.float32
F8 = mybir.dt.float8e4
BF16 = mybir.dt.bfloat16
E4NP = ml_dtypes.float8_e4m3
DR = mybir.MatmulPerfMode.DoubleRow

_CACHE = {}


def build():
    nc = bacc.Bacc(None, target_bir_lowering=False)

    eh_d = nc.dram_tensor("eh", [H, R], F8, kind="ExternalInput")
    el_d = nc.dram_tensor("el", [H, R], F8, kind="ExternalInput")
    wh_d = nc.dram_tensor("wh", [H, H], F8, kind="ExternalInput")
    wl_d = nc.dram_tensor("wl", [H, H], F8, kind="ExternalInput")
    cov_d = nc.dram_tensor("cov", [1, R], F32, kind="ExternalInput")
    wcov_d = nc.dram_tensor("wcov", [P, MC], F32, kind="ExternalInput")
    v_d = nc.dram_tensor("v", [P, MC], F32, kind="ExternalInput")
    dec_d = nc.dram_tensor("dec", [P, MC, BC], F32, kind="ExternalInput")
    out_d = nc.dram_tensor("attn", [BC, S], F32, kind="ExternalOutput")

    with tile.TileContext(nc) as tc:
        with (
            tc.tile_pool(name="const", bufs=1) as const,
            tc.tile_pool(name="stream", bufs=3) as stream,
            tc.tile_pool(name="bcp", bufs=2) as bcp,
            tc.tile_pool(name="ttp", bufs=8) as ttp,
            tc.tile_pool(name="vtp", bufs=2) as vtp,
            tc.tile_pool(name="scp", bufs=2) as scp,
            tc.tile_pool(name="sm", bufs=2) as smp,
            tc.tile_pool(name="psm", bufs=6, space="PSUM") as psm,
        ):
            wh_sb = const.tile([P, KC, H], F8)
            wl_sb = const.tile([P, KC, H], F8)
            eh0 = stream.tile([P, KC, NF], F8, tag="eh")
            el0 = stream.tile([P, KC, NF], F8, tag="el")
            wcov_sb = const.tile([P, MC], F32)
            v_sb = const.tile([P, MC], F32)
            dec_sb = const.tile([P, MC, BC], F32)
            cov_sb = const.tile([1, R], F32)
            wup = const.tile([P, MC], F8)

            # Warmup source must be initialized before the PE touches it.
            nc.vector.memset(wup[:], 0.0)

            # First-needed-first DMA issue. The sync (SP) queue carries the
            # first enc tiles and Wh; ACT's queue carries el/Wl in parallel;
            # DVE's queue carries the small constants.
            nc.sync.dma_start(
                eh0[:], eh_d.ap()[:, 0:NF].rearrange("(k p) r -> p k r", p=P)
            )
            nc.sync.dma_start(
                wh_sb[:, :, 0:P],
                wh_d.ap()[:, 0:P].rearrange("(k p) c -> p k c", p=P),
            )
            nc.scalar.dma_start(
                el0[:], el_d.ap()[:, 0:NF].rearrange("(k p) r -> p k r", p=P)
            )
            nc.scalar.dma_start(
                wl_sb[:, :, 0:P],
                wl_d.ap()[:, 0:P].rearrange("(k p) c -> p k c", p=P),
            )
            nc.scalar.dma_start(v_sb[:], v_d.ap())
            nc.scalar.dma_start(wcov_sb[:], wcov_d.ap())
            nc.scalar.dma_start(cov_sb[:], cov_d.ap())
            nc.scalar.dma_start(dec_sb[:], dec_d.ap())

            # PE warmup: tiny matmuls fill the initial DMA wait so the PE
            # p-state clock is ramped for the real matmul stream.
            wpsum = psm.tile([P, NF], F32, tag="pm")
            for _ in range(300):
                nc.tensor.matmul(
                    wpsum[0:MC, 0:MC], wup[:], wup[:], start=True, stop=True
                )

            for mc in range(1, MC):
                cs = slice(mc * P, (mc + 1) * P)
                nc.sync.dma_start(
                    wh_sb[:, :, cs],
                    wh_d.ap()[:, cs].rearrange("(k p) c -> p k c", p=P),
                )
                nc.scalar.dma_start(
                    wl_sb[:, :, cs],
                    wl_d.ap()[:, cs].rearrange("(k p) c -> p k c", p=P),
                )

            ex = None
            psums = None
            for rb in range(NRB):
                b = rb // RB_PER_B
                i = rb % RB_PER_B
                so = i * NF
                r0 = rb * NF

                if rb == 0:
                    eh, el = eh0, el0
                else:
                    eh = stream.tile([P, KC, NF], F8, tag="eh")
                    nc.sync.dma_start(
                        eh[:],
                        eh_d.ap()[:, r0 : r0 + NF].rearrange(
                            "(k p) r -> p k r", p=P
                        ),
                    )
                    el = stream.tile([P, KC, NF], F8, tag="el")
                    nc.sync.dma_start(
                        el[:],
                        el_d.ap()[:, r0 : r0 + NF].rearrange(
                            "(k p) r -> p k r", p=P
                        ),
                    )

                # coverage slice broadcast to all 128 partitions (gpsimd)
                bc = bcp.tile([P, NF], F32, tag="bc")
                nc.gpsimd.partition_broadcast(bc[:], cov_sb[:, r0 : r0 + NF])

                if i == 0:
                    ex = smp.tile([1, S], F32, tag="ex")
                    psums = smp.tile([1, RB_PER_B], F32, tag="psums")

                vt = vtp.tile([P, NF], F32, tag="vt")

                for m in range(MC):
                    ms = slice(m * P, (m + 1) * P)
                    pm = psm.tile([P, NF], F32, tag="pm")
                    # 12 DoubleRow matmuls: eh@Wh, eh@Wl, el@Wh — one PSUM
                    # accumulation group. el-dependent passes go last so the
                    # first row-block can start before el lands.
                    for k2 in range(KC // 2):
                        ks = slice(2 * k2, 2 * k2 + 2)
                        nc.tensor.matmul(
                            pm[:],
                            wh_sb[:, ks, ms],
                            eh[:, ks, :],
                            start=(k2 == 0),
                            stop=False,
                            perf_mode=DR,
                        )
                    for k2 in range(KC // 2):
                        ks = slice(2 * k2, 2 * k2 + 2)
                        nc.tensor.matmul(
                            pm[:],
                            wl_sb[:, ks, ms],
                            eh[:, ks, :],
                            start=False,
                            stop=False,
                            perf_mode=DR,
                        )
                    for k2 in range(KC // 2):
                        ks = slice(2 * k2, 2 * k2 + 2)
                        nc.tensor.matmul(
                            pm[:],
                            wh_sb[:, ks, ms],
                            el[:, ks, :],
                            start=False,
                            stop=(k2 == KC // 2 - 1),
                            perf_mode=DR,
                        )
                    # cov rank-1 term fused into PSUM in place:
                    # pm = bc * wcov[:,m] + pm   (DVE, one instruction)
                    nc.vector.scalar_tensor_tensor(
                        pm[:],
                        bc[:],
                        wcov_sb[:, m : m + 1],
                        pm[:],
                        mybir.AluOpType.mult,
                        mybir.AluOpType.add,
                    )
                    tt = ttp.tile([P, NF], BF16, tag="tt")
                    nc.scalar.activation(
                        tt[:],
                        pm[:],
                        mybir.ActivationFunctionType.Tanh,
                        bias=dec_sb[:, m, b : b + 1],
                        scale=1.0 / ALPHA,
                    )
                    # v-dot accumulation on DVE: vt += tt * v[:,m]
                    # (walrus rejects TensorScalarPtr on Pool, so the whole
                    # chain lives on DVE; DVE stays just under the PE cadence)
                    if m == 0:
                        nc.vector.tensor_scalar_mul(
                            vt[:], tt[:], v_sb[:, m : m + 1]
                        )
                    else:
                        nc.vector.scalar_tensor_tensor(
                            vt[:],
                            tt[:],
                            v_sb[:, m : m + 1],
                            vt[:],
                            mybir.AluOpType.mult,
                            mybir.AluOpType.add,
                        )

                sc = scp.tile([P, NF], F32, tag="sc")
                nc.gpsimd.partition_all_reduce(
                    sc[:], vt[:], P, bass_isa.ReduceOp.add
                )
                # exp with the ACT accumulator emitting this block's partial
                # sum directly (keeps the per-rb reduce off DVE)
                nc.scalar.activation(
                    ex[:, so : so + NF],
                    sc[0:1, :],
                    mybir.ActivationFunctionType.Exp,
                    accum_out=psums[:, i : i + 1],
                )

                if i == RB_PER_B - 1:
                    ssum = smp.tile([1, 1], F32, tag="ssum")
                    nc.vector.reduce_sum(
                        ssum[:], psums[:], axis=mybir.AxisListType.X
                    )
                    rsum = smp.tile([1, 1], F32, tag="rsum")
                    nc.vector.reciprocal(rsum[:], ssum[:])
                    ob = smp.tile([1, S], F32, tag="ob")
                    # normalize on ACT (Copy activation with per-partition
                    # scale) — keeps DVE under the PE cadence
                    if rb == NRB - 1:
                        # halve the last normalize so the tail pipelines
                        HS = S // 2
                        nc.scalar.activation(
                            ob[:, 0:HS],
                            ex[:, 0:HS],
                            mybir.ActivationFunctionType.Copy,
                            scale=rsum[:],
                        )
                        nc.scalar.dma_start(
                            out_d.ap()[b : b + 1, 0:HS], ob[:, 0:HS]
                        )
                        nc.scalar.activation(
                            ob[:, HS:S],
                            ex[:, HS:S],
                            mybir.ActivationFunctionType.Copy,
                            scale=rsum[:],
                        )
                        nc.sync.dma_start(
                            out_d.ap()[b : b + 1, HS:S], ob[:, HS:S]
                        )
                    else:
                        nc.scalar.activation(
                            ob[:],
                            ex[:],
                            mybir.ActivationFunctionType.Copy,
                            scale=rsum[:],
                        )
                        nc.scalar.dma_start(out_d.ap()[b : b + 1, :], ob[:])

    nc.compile()
    return nc


def _get_nc():
    if "nc" not in _CACHE:
        _CACHE["nc"] = build()
    return _CACHE["nc"]


def prep_in_maps(decoder_hidden, encoder_outputs, coverage, W_enc, W_dec, b_dec, w_cov, v):
    decoder_hidden = np.asarray(decoder_hidden, dtype=np.float32)
    encoder_outputs = np.asarray(encoder_outputs, dtype=np.float32)
    coverage = np.asarray(coverage, dtype=np.float32)
    W_enc = np.asarray(W_enc, dtype=np.float32)
    W_dec = np.asarray(W_dec, dtype=np.float32)
    b_dec = np.asarray(b_dec, dtype=np.float32)
    w_cov = np.asarray(w_cov, dtype=np.float32)
    v = np.asarray(v, dtype=np.float32)

    # host-side tiny matmul: dec_feature [B, H]
    dec_feature = decoder_hidden[:, 0, :] @ W_dec.T + b_dec

    # W.T scaled by 32 (exact power of 2), split hi/lo into e4m3 with error
    # feedback. The x32 keeps Wl out of fp8 subnormal underflow.
    w32 = np.ascontiguousarray(W_enc.T) * np.float32(ALPHA)   # [H(in), H(out)]
    wh8 = w32.astype(E4NP)
    wl8 = (w32 - wh8.astype(np.float32)).astype(E4NP)
    wcov_r = np.ascontiguousarray(
        (w_cov * np.float32(ALPHA)).reshape(MC, P).T
    )                                                         # [P, MC]
    v_r = np.ascontiguousarray(v.reshape(MC, P).T)            # [P, MC] f32

    in_maps = []
    for c in range(NCORES):
        bs = slice(c * BC, (c + 1) * BC)
        encT = np.ascontiguousarray(
            encoder_outputs[bs].reshape(R, H).T               # [H, R]
        )
        eh8 = encT.astype(E4NP)
        el8 = (encT - eh8.astype(np.float32)).astype(E4NP)
        cov = np.ascontiguousarray(coverage[bs].reshape(1, R))
        dec = np.ascontiguousarray(
            dec_feature[bs].T.reshape(MC, P, BC).transpose(1, 0, 2)  # [P, MC, BC]
        )
        in_maps.append(
            {
                "eh": eh8,
                "el": el8,
                "wh": wh8,
                "wl": wl8,
                "cov": cov,
                "wcov": wcov_r,
                "v": v_r,
                "dec": dec,
            }
        )
    return in_maps


def kernel(decoder_hidden, encoder_outputs, coverage, W_enc, W_dec, b_dec, w_cov, v):
    nc = _get_nc()
    in_maps = prep_in_maps(
        decoder_hidden, encoder_outputs, coverage, W_enc, W_dec, b_dec, w_cov, v
    )
    res = run_bass_kernel_spmd(nc, in_maps, core_ids=list(range(NCORES)))
    out = np.concatenate([r["attn"] for r in res.results], axis=0)  # [B, S]
    return out[:, None, :].astype(np.float32)                       # [B, 1, S]
